# revision 7
# baseline (speedup 1.0000x reference)
"""GCN encoder (sigmoid gate + 2x GCNConv) on 8 Trainium2 NeuronCores.

Strategy (SPMD, one program on 8 cores):
  - Nodes are sharded contiguously (12500 rows/core) for the OUTPUT; edges are
    assigned to the core owning their destination.  Self loops are ordinary
    edges.  Weight matrices are replicated.
  - deg/dinv are graph metadata computed on host.  dinv is folded into the
    data path: the kernel receives both xT and (dinv*x)T, tables store
    g~ = dinv_src * (h @ W), and dinv_dst is applied at the window flush, so
    no per-edge norm values exist on device.
  - Layer-1 table: every core redundantly computes the FULL g~1 table into
    local DRAM (dense phase over all 100k nodes, row-major via
    matmul(lhsT=h0_tile, rhs=W)), laid out in the same 4x25000-row block
    order an AllGather would produce.  No layer-1 collective.
  - Layer-2 table: computed shard-locally from h~1 and AllGathered in 4
    chunks (fired as soon as the producing windows flush, overlapping the
    tail of layer-1's sparse phase and layer-2's first pass).
  - Sparse phase per layer: edges grouped by (dst window of 128, src block
    of 25000); source rows fetched with dma_gather (int16 indices), calls
    round-robined over SWDGE queues 0-3 so descriptor generation runs on
    all four Q7 core pairs in parallel (3.4x measured).
  - Scatter: one-hot S[e, slot] built 32 tiles at a time by a single
    tensor_tensor is_equal with broadcast access patterns (edst vs iota),
    alternating Vector/Scalar engines; matmul(lhsT=S_tile, rhs=gathered)
    accumulates [slot, feat] (row-major) windows in PSUM, so the final
    output needs no transpose.

The harness calls kernel(**inputs) with full-size inputs; everything below
is self-contained (no file reads).
"""

import math
import os

import numpy as np

import concourse.bacc as bacc
import concourse.bass as bass
import concourse.mybir as mybir
import concourse.tile as tile
from concourse import library_config
from concourse.bass_utils import run_bass_kernel_spmd
from concourse.masks import make_identity

F32 = mybir.dt.float32
F16 = mybir.dt.float16
I16 = mybir.dt.int16

N_CORES = 8
D = 128  # feature dim == hidden dim == partition count

LAST_RESULTS = None  # set by kernel(); lets a test harness grab the results
LAST_NC = None       # compiled Bass module of the last kernel() call
LAST_IN_MAPS = None  # per-core input dicts of the last kernel() call
LAST_META = None     # sharding metadata of the last kernel() call


# --------------------------------------------------------------------------
# host-side sharding / metadata
# --------------------------------------------------------------------------

class Meta:
    pass


def _prep(x, edge_index, gate_W, gate_b, W1, b1, W2, b2,
          n_cores=N_CORES, win=128, nblk=4, tq=32):
    """Shard inputs, group edges, build per-core device input dicts plus the
    (core-independent) program structure metadata."""
    x = np.asarray(x, np.float32)
    N, d = x.shape
    assert d == D
    src = np.asarray(edge_index[0]).astype(np.int64)
    dst = np.asarray(edge_index[1]).astype(np.int64)

    nloc = N // n_cores
    assert nloc * n_cores == N
    assert nloc % nblk == 0
    blk_sub = nloc // nblk          # rows each core contributes to a block
    blk_rows = blk_sub * n_cores    # rows of one table block
    assert blk_rows < 32768, "dma_gather idx is int16"
    nwin = math.ceil(nloc / win)

    deg = np.bincount(dst, minlength=N).astype(np.float64) + 1.0
    dinv = (1.0 / np.sqrt(deg)).astype(np.float32)

    loop = np.arange(N, dtype=np.int64)
    s_all = np.concatenate([src, loop])
    d_all = np.concatenate([dst, loop])

    # src -> (block, row inside block); block k holds rows
    # [k*blk_sub, (k+1)*blk_sub) of every core's shard, in rank order.
    s_core = s_all // nloc
    s_rem = s_all % nloc
    s_blk = s_rem // blk_sub
    s_idx = (s_core * blk_sub + s_rem % blk_sub).astype(np.int64)

    e_core = d_all // nloc
    ld = d_all % nloc
    e_win = ld // win
    e_slot = ld % win

    # tiles per (window, block): max over cores so the program is identical
    key = ((e_core * nwin + e_win) * nblk + s_blk).astype(np.int64)
    cnt = np.bincount(key, minlength=n_cores * nwin * nblk)
    cnt = cnt.reshape(n_cores, nwin, nblk)
    T_wb = -(-cnt.max(axis=0) // 128)           # [nwin, nblk]
    assert (T_wb[:, :2].sum(axis=1) > 0).all()
    assert (T_wb[:, 2:].sum(axis=1) > 0).all()

    tstart = np.zeros((nwin, nblk), np.int64)
    tstart[1:, :] = np.cumsum(T_wb[:-1, :], axis=0)
    blk_tiles = T_wb.sum(axis=0)                # [nblk]
    blk_off = np.concatenate([[0], np.cumsum(blk_tiles)])
    ntiles_tot = int(blk_off[-1])

    calls_blk = [int(math.ceil(blk_tiles[b] / tq)) for b in range(nblk)]
    icols_blk = [calls_blk[b] * tq * 8 for b in range(nblk)]
    icol_off = np.concatenate([[0], np.cumsum(icols_blk)]).astype(np.int64)
    icols_tot = int(icol_off[-1])

    m = Meta()
    m.n_cores, m.win, m.nblk, m.tq = n_cores, win, nblk, tq
    m.nloc, m.blk_sub, m.blk_rows, m.nwin = nloc, blk_sub, blk_rows, nwin
    m.N = N
    m.T_wb, m.tstart = T_wb, tstart
    m.blk_tiles, m.blk_off = blk_tiles, blk_off
    m.calls_blk, m.icol_off = calls_blk, icol_off
    m.ntiles_tot, m.icols_tot = ntiles_tot, icols_tot
    m.passes = [[0, 1], [2, 3]]

    gw = np.asarray(gate_W, np.float16)
    w1 = np.asarray(W1, np.float16)
    w2 = np.asarray(W2, np.float16)
    gb = np.asarray(gate_b, np.float32).reshape(D, 1)
    b1rep = np.tile(np.asarray(b1, np.float32).reshape(1, D), (128, 1))
    b2rep = np.tile(np.asarray(b2, np.float32).reshape(1, D), (128, 1))

    xT = np.ascontiguousarray(x.T.astype(np.float16))
    xdT = np.ascontiguousarray((x * dinv[:, None]).T.astype(np.float16))

    in_maps = []
    for c in range(n_cores):
        sel = np.nonzero(e_core == c)[0]
        eb = s_blk[sel]
        ew = e_win[sel]
        order = np.lexsort((ew, eb))
        sel = sel[order]
        eb = eb[order]
        ew = ew[order]
        es = s_idx[sel]
        eslot = e_slot[sel]

        gkey = eb * nwin + ew
        group_start = np.searchsorted(gkey, np.arange(nblk * nwin))
        rank = np.arange(len(gkey)) - group_start[gkey]
        tg = rank // 128
        p = rank % 128
        bt = tstart[ew, eb] + tg                 # tile index inside block
        col = blk_off[eb] + bt                   # global meta column
        assert (tg < T_wb[ew, eb]).all()

        edst = np.full((128, ntiles_tot), -1.0, np.float16)
        edst[p, col] = eslot.astype(np.float16)

        idx_cols = []
        for b in range(nblk):
            mask_b = eb == b
            flat = np.zeros(calls_blk[b] * tq * 128, np.int16)
            flat[(bt[mask_b] * 128 + p[mask_b])] = es[mask_b].astype(np.int16)
            for cidx in range(calls_blk[b]):
                v = flat[cidx * tq * 128:(cidx + 1) * tq * 128]
                idx_cols.append(v.reshape(tq * 8, 16).T)
        idx16 = np.concatenate(idx_cols, axis=1)
        assert idx16.shape == (16, icols_tot)
        idx16 = np.tile(idx16, (8, 1))

        dv = np.zeros((128, nwin), np.float32)
        dloc = dinv[c * nloc:(c + 1) * nloc]
        for w in range(nwin):
            s = dloc[w * win:(w + 1) * win]
            dv[:len(s), w] = s

        in_maps.append({
            "xT": xT, "xdT": xdT,
            "gw": gw, "gbias": gb, "w1": w1, "b1rep": b1rep,
            "w2": w2, "b2rep": b2rep,
            "dinvw": dv,
            "eidx": np.ascontiguousarray(idx16),
            "edst": edst,
        })
    return in_maps, m


# --------------------------------------------------------------------------
# device program
# --------------------------------------------------------------------------

def _emit(tc, outs, ins, m):
    nc = tc.nc
    AG = mybir.AluOpType
    AF = mybir.ActivationFunctionType
    groups = [list(range(m.n_cores))]
    out_ap = outs["out"]

    def span(w):
        return min(m.win, m.nloc - w * m.win)

    with (
        tc.tile_pool(name="sb", bufs=1) as sb,
        tc.tile_pool(name="ps", bufs=1, space="PSUM") as ps,
        tc.tile_pool(name="dr", bufs=1, space="DRAM") as dr,
    ):
        nc.gpsimd.load_library(library_config.mlp)

        # ---- constants / params ------------------------------------------
        ident16 = sb.tile([128, 128], F16, tag="id16")
        make_identity(nc, ident16[:, :])
        iota16 = sb.tile([128, 128], F16, tag="iota")
        nc.gpsimd.iota(iota16[:, :], pattern=[[1, 128]], base=0,
                       channel_multiplier=0,
                       allow_small_or_imprecise_dtypes=True)

        wgate = sb.tile([128, 128], F16, tag="wgate")
        nc.sync.dma_start(wgate[:, :], ins["gw"][:, :])
        w1sb = sb.tile([128, 128], F16, tag="w1sb")
        nc.sync.dma_start(w1sb[:, :], ins["w1"][:, :])
        w2sb = sb.tile([128, 128], F16, tag="w2sb")
        nc.sync.dma_start(w2sb[:, :], ins["w2"][:, :])
        gbias = sb.tile([128, 1], F32, tag="gbias")
        nc.sync.dma_start(gbias[:, :], ins["gbias"][:, :])
        b1rep = sb.tile([128, 128], F32, tag="b1rep")
        nc.sync.dma_start(b1rep[:, :], ins["b1rep"][:, :])
        b2rep = sb.tile([128, 128], F32, tag="b2rep")
        nc.sync.dma_start(b2rep[:, :], ins["b2rep"][:, :])
        dinvw = sb.tile([128, m.nwin], F32, tag="dinvw")
        nc.sync.dma_start(dinvw[:, :], ins["dinvw"][:, :])

        # ---- resident edge metadata --------------------------------------
        dst_sb = sb.tile([128, m.ntiles_tot], F16, tag="dst_sb")
        nc.sync.dma_start(dst_sb[:, :], ins["edst"][:, :])

        h1T = sb.tile([128, m.nloc], F16, tag="h1T")
        accT = sb.tile([128, m.nwin, 128], F32, tag="accT")

        # ---- DRAM scratch -------------------------------------------------
        l1blk = [dr.tile([m.blk_rows, 128], F16, tag=f"l1blk{k}",
                          name=f"l1blk{k}")
                 for k in range(m.nblk)]
        g2_loc = dr.tile([m.nloc, 128], F16, tag="g2_loc")
        g2_full = [dr.tile([m.blk_rows, 128], F16, tag=f"g2_full{k}",
                           name=f"g2_full{k}", addr_space="Shared")
                   for k in range(m.nblk)]

        # ---- phase A: full g~1 table, row-major, block-major order -------
        CH = 512
        for b in range(m.nblk):
            for cs in range(m.n_cores):
                cbase = cs * m.nloc + b * m.blk_sub
                rbase = cs * m.blk_sub
                for off in range(0, m.blk_sub, CH):
                    cc = min(CH, m.blk_sub - off)
                    nsub = math.ceil(cc / 128)
                    xt = sb.tile([128, CH], F16, tag="xt", bufs=3)
                    nc.sync.dma_start(xt[:, :cc],
                                      ins["xT"][:, cbase + off:cbase + off + cc])
                    xdt = sb.tile([128, CH], F16, tag="xdt", bufs=3)
                    nc.sync.dma_start(xdt[:, :cc],
                                      ins["xdT"][:, cbase + off:cbase + off + cc])
                    pg = ps.tile([128, CH], F32, tag="pg", bufs=2)
                    nc.tensor.matmul(pg[:, :cc], lhsT=wgate[:, :],
                                     rhs=xt[:, :cc], start=True, stop=True)
                    gt = sb.tile([128, CH], F16, tag="gt", bufs=2)
                    nc.scalar.activation(gt[:, :cc], pg[:, :cc], AF.Sigmoid,
                                         bias=gbias[:, :])
                    h0 = sb.tile([128, CH], F16, tag="h0", bufs=2)
                    nc.vector.tensor_tensor(out=h0[:, :cc], in0=xdt[:, :cc],
                                            in1=gt[:, :cc], op=AG.mult)
                    rp = ps.tile([128, nsub, 128], F32, tag="rp", bufs=2)
                    for s in range(nsub):
                        sw = min(128, cc - s * 128)
                        nc.tensor.matmul(rp[:sw, s, :],
                                         lhsT=h0[:, s * 128:s * 128 + sw],
                                         rhs=w1sb[:, :], start=True, stop=True)
                    rc = sb.tile([128, nsub, 128], F16, tag="rc", bufs=2)
                    if cc == CH:
                        nc.scalar.copy(rc[:, :, :], rp[:, :, :])
                        nc.sync.dma_start(
                            l1blk[b][rbase + off:rbase + off + cc, :]
                            .rearrange("(t p) f -> p t f", p=128),
                            rc[:, :, :])
                    else:
                        for s in range(nsub):
                            sw = min(128, cc - s * 128)
                            nc.scalar.copy(rc[:sw, s, :], rp[:sw, s, :])
                            nc.sync.dma_start(
                                l1blk[b][rbase + off + s * 128:
                                         rbase + off + s * 128 + sw, :],
                                rc[:sw, s, :])

        # ---- sparse phase ------------------------------------------------
        IGRP = 4  # idx cols loaded per DMA, in units of gather calls
        qctr = [0]   # round-robin SWDGE queue counter (shared both layers)
        sctr = [0]   # S-build engine alternation counter

        def spmm(tables, flush):
            gbufs = {}
            sbufs = {}
            idxbufs = {}

            def idx_slice(b, call):
                grp = call // IGRP
                if (b, grp) not in idxbufs:
                    ic0 = int(m.icol_off[b]) + grp * IGRP * m.tq * 8
                    cols = min(IGRP * m.tq * 8,
                               int(m.icol_off[b + 1]) - ic0)
                    buf = sb.tile([128, IGRP * m.tq * 8], I16, tag="idxb",
                                  bufs=4, name=f"idxb{b}_{grp}")
                    nc.sync.dma_start(buf[:, :cols],
                                      ins["eidx"][:, ic0:ic0 + cols])
                    idxbufs[(b, grp)] = buf
                off = (call % IGRP) * m.tq * 8
                return idxbufs[(b, grp)], off

            def ensure_call(b, call):
                if (b, call) in gbufs:
                    return
                ntile = int(min(m.tq, m.blk_tiles[b] - call * m.tq))
                gbuf = sb.tile([128, m.tq, 128], F16, tag="gbuf",
                               bufs=4, name=f"gbuf{b}_{call}")
                nidx = ntile * 128
                ibuf, ioff = idx_slice(b, call)
                nc.gpsimd.dma_gather(
                    gbuf[:, :ntile, :], tables[b][:, :],
                    ibuf[:, ioff:ioff + ntile * 8], nidx, nidx, 128,
                    single_packet=(nidx * 2 <= 4096),
                    queue_num=qctr[0] % 4)
                qctr[0] += 1
                # one-hot S for all tiles of this call:
                # S[p, t, col] = (edst[p, t0+t] == iota[col])
                t0 = int(m.blk_off[b]) + call * m.tq
                sbuf = sb.tile([128, m.tq, 128], F16, tag="sbuf",
                               bufs=4, name=f"sbuf{b}_{call}")
                sctr[0] += 1
                nc.vector.tensor_tensor(
                    out=sbuf[:, :ntile, :],
                    in0=dst_sb[:, t0:t0 + ntile].unsqueeze(2)
                        .broadcast_to([128, ntile, 128]),
                    in1=iota16[:, :].unsqueeze(1)
                        .broadcast_to([128, ntile, 128]),
                    op=AG.is_equal)
                gbufs[(b, call)] = gbuf
                sbufs[(b, call)] = sbuf

            for p, blocks in enumerate(m.passes):
                for w in range(m.nwin):
                    nmm = sum(int(m.T_wb[w, b]) for b in blocks)
                    if nmm == 0:
                        continue
                    cols = span(w)
                    for b in blocks:
                        if m.T_wb[w, b] == 0:
                            continue
                        t0 = int(m.tstart[w, b])
                        t1 = t0 + int(m.T_wb[w, b])
                        for call in range(t0 // m.tq, (t1 - 1) // m.tq + 1):
                            ensure_call(b, call)
                    psw = ps.tile([128, 128], F32, tag="win", bufs=2)
                    k = 0
                    for b in blocks:
                        t0 = int(m.tstart[w, b])
                        for t in range(int(m.T_wb[w, b])):
                            bt = t0 + t
                            call = bt // m.tq
                            ti = bt % m.tq
                            nc.tensor.matmul(
                                psw[:cols, :],
                                lhsT=sbufs[(b, call)][:, ti, :cols],
                                rhs=gbufs[(b, call)][:, ti, :],
                                start=(k == 0), stop=(k == nmm - 1))
                            k += 1
                    dv = dinvw[:cols, w:w + 1]
                    if p == 0:
                        nc.vector.tensor_scalar(
                            accT[:cols, w, :], psw[:cols, :], dv, None,
                            op0=AG.mult)
                    else:
                        flush(w, cols, psw, dv)

            # windows whose pass-1 half is empty (never happens with self
            # loops present, asserted host-side)

        # ---- layer 1 flush: h1 = relu(dinv*sum + b1); h~1 = dinv*h1 ------
        # interleaved layer-2 dense + chunked AllGather
        ag_after = {}
        for k in range(m.nblk):
            ag_after.setdefault(((k + 1) * m.blk_sub - 1) // m.win,
                                []).append(k)

        def flush1(w, cols, psw, dv):
            t1 = sb.tile([128, 128], F32, tag="fl_t1", bufs=2)
            nc.vector.scalar_tensor_tensor(
                out=t1[:cols, :], in0=psw[:cols, :], scalar=dv,
                in1=accT[:cols, w, :], op0=AG.mult, op1=AG.add)
            t2 = sb.tile([128, 128], F32, tag="fl_t2", bufs=2)
            nc.vector.tensor_tensor(out=t2[:cols, :], in0=t1[:cols, :],
                                    in1=b1rep[:cols, :], op=AG.add)
            hrow = sb.tile([128, 128], F16, tag="hrow", bufs=2)
            nc.vector.tensor_scalar(hrow[:cols, :], t2[:cols, :], dv, 0.0,
                                    op0=AG.mult, op1=AG.max)
            # transpose to feature-major h~1T for the layer-2 dense matmul
            tp = ps.tile([128, 128], F16, tag="tp", bufs=1)
            nc.tensor.transpose(tp[:, :cols], hrow[:cols, :],
                                ident16[:cols, :cols])
            nc.scalar.copy(h1T[:, w * m.win:w * m.win + cols], tp[:, :cols])
            # layer-2 dense: g~2 rows for this window
            p2 = ps.tile([128, 128], F32, tag="p2", bufs=1)
            nc.tensor.matmul(p2[:cols, :],
                             lhsT=h1T[:, w * m.win:w * m.win + cols],
                             rhs=w2sb[:, :], start=True, stop=True)
            g2c = sb.tile([128, 128], F16, tag="g2c", bufs=2)
            nc.scalar.copy(g2c[:cols, :], p2[:cols, :])
            nc.sync.dma_start(g2_loc[w * m.win:w * m.win + cols, :],
                              g2c[:cols, :])
            for k in ag_after.get(w, []):
                nc.gpsimd.collective_compute(
                    "AllGather", AG.bypass, replica_groups=groups,
                    ins=[g2_loc[k * m.blk_sub:(k + 1) * m.blk_sub, :]],
                    outs=[g2_full[k][:, :]],
                )

        spmm(l1blk, flush1)

        # ---- layer 2 sparse + final flush --------------------------------
        def flush2(w, cols, psw, dv):
            t1 = sb.tile([128, 128], F32, tag="f2_t1", bufs=2)
            nc.vector.scalar_tensor_tensor(
                out=t1[:cols, :], in0=psw[:cols, :], scalar=dv,
                in1=accT[:cols, w, :], op0=AG.mult, op1=AG.add)
            t2 = sb.tile([128, 128], F32, tag="f2_t2", bufs=2)
            nc.vector.tensor_tensor(out=t2[:cols, :], in0=t1[:cols, :],
                                    in1=b2rep[:cols, :], op=AG.add)
            nc.sync.dma_start(out_ap[w * m.win:w * m.win + cols, :],
                              t2[:cols, :])

        spmm(g2_full, flush2)


def declare_io(nc, m):
    ins = {
        "xT": nc.dram_tensor("xT", [D, m.N], F16, kind="ExternalInput").ap(),
        "xdT": nc.dram_tensor("xdT", [D, m.N], F16, kind="ExternalInput").ap(),
        "gw": nc.dram_tensor("gw", [D, D], F16, kind="ExternalInput").ap(),
        "gbias": nc.dram_tensor("gbias", [D, 1], F32, kind="ExternalInput").ap(),
        "w1": nc.dram_tensor("w1", [D, D], F16, kind="ExternalInput").ap(),
        "b1rep": nc.dram_tensor("b1rep", [D, D], F32, kind="ExternalInput").ap(),
        "w2": nc.dram_tensor("w2", [D, D], F16, kind="ExternalInput").ap(),
        "b2rep": nc.dram_tensor("b2rep", [D, D], F32, kind="ExternalInput").ap(),
        "dinvw": nc.dram_tensor("dinvw", [D, m.nwin], F32,
                                kind="ExternalInput").ap(),
        "eidx": nc.dram_tensor("eidx", [128, max(m.icols_tot, 1)], I16,
                               kind="ExternalInput").ap(),
        "edst": nc.dram_tensor("edst", [128, max(m.ntiles_tot, 1)], F16,
                               kind="ExternalInput").ap(),
    }
    outs = {
        "out": nc.dram_tensor("out", [m.nloc, D], F32,
                              kind="ExternalOutput").ap(),
    }
    return ins, outs


def _build(m):
    nc = bacc.Bacc("TRN2", target_bir_lowering=False, debug=False,
                   enable_asserts=False, num_devices=m.n_cores,
                   num_swdge_queues=4)
    ins, outs = declare_io(nc, m)
    with tile.TileContext(nc) as tc:
        _emit(tc, outs, ins, m)
    nc.compile()
    return nc


def kernel(**inputs):
    global LAST_RESULTS, LAST_NC, LAST_IN_MAPS, LAST_META
    in_maps, m = _prep(**inputs)
    nc = _build(m)
    LAST_NC, LAST_IN_MAPS, LAST_META = nc, in_maps, m
    res = run_bass_kernel_spmd(
        nc, in_maps, core_ids=list(range(m.n_cores)), trace=False)
    LAST_RESULTS = res
    out = np.concatenate([res.results[c]["out"] for c in range(m.n_cores)],
                         axis=0)
    return np.ascontiguousarray(out.astype(np.float32))


# revision 9
# speedup vs baseline: 2.0455x; 2.0455x over previous
"""GCN encoder (sigmoid gate + 2x GCNConv) on 8 Trainium2 NeuronCores.

Strategy (SPMD, one program on 8 cores):
  - Nodes are sharded contiguously (12500 rows/core); edges are assigned to
    the core owning their destination.  Self loops are ordinary edges.
    Weight matrices are replicated.
  - deg/dinv are graph metadata computed on host.  dinv is folded into the
    data path: the kernel receives both xT and (dinv*x)T shards, tables
    store g~ = dinv_src * (h @ W), and dinv_dst is applied at the window
    flush, so no per-edge norm values exist on device.
  - Per layer: a sharded dense phase produces the local table rows
    row-major (matmul(lhsT=h_tile, rhs=W) directly yields [node, feat])
    which are AllGathered in 4 chunks; the sparse phase chases the chunks
    in two passes over block pairs {0,1}, {2,3}.
  - Sparse phase: edges grouped by (dst window of 128, src block of 25000
    AllGather-layout rows, same grouping both layers); source rows fetched
    with dma_gather (int16 indices), calls round-robined over SWDGE queues
    0-3 so descriptor generation runs on all four Q7 core pairs in
    parallel (3.4x measured).
  - Scatter: one-hot S[e, slot] built 32 tiles at a time by a single
    tensor_tensor is_equal with broadcast access patterns (edst vs iota);
    matmul(lhsT=S_tile, rhs=gathered) accumulates [slot, feat] (row-major)
    windows in PSUM, so the final output needs no transpose.
  - Layer-1 flush chains into the layer-2 dense + chunked AllGather with a
    few-window lag so PE stage-2 work never blocks the sparse stream.

The harness calls kernel(**inputs) with full-size inputs; everything below
is self-contained (no file reads).
"""

import math
import os

import numpy as np

import concourse.bacc as bacc
import concourse.bass as bass
import concourse.mybir as mybir
import concourse.tile as tile
from concourse import library_config
from concourse.bass_utils import run_bass_kernel_spmd
from concourse.masks import make_identity

F32 = mybir.dt.float32
F16 = mybir.dt.float16
I16 = mybir.dt.int16

N_CORES = 8
D = 128  # feature dim == hidden dim == partition count

LAST_RESULTS = None  # set by kernel(); lets a test harness grab the results
LAST_NC = None       # compiled Bass module of the last kernel() call
LAST_IN_MAPS = None  # per-core input dicts of the last kernel() call
LAST_META = None     # sharding metadata of the last kernel() call


# --------------------------------------------------------------------------
# host-side sharding / metadata
# --------------------------------------------------------------------------

class Meta:
    pass


def _prep(x, edge_index, gate_W, gate_b, W1, b1, W2, b2,
          n_cores=N_CORES, win=128, nblk=4, tq=32):
    """Shard inputs, group edges, build per-core device input dicts plus the
    (core-independent) program structure metadata."""
    x = np.asarray(x, np.float32)
    N, d = x.shape
    assert d == D
    src = np.asarray(edge_index[0]).astype(np.int64)
    dst = np.asarray(edge_index[1]).astype(np.int64)

    nloc = N // n_cores
    assert nloc * n_cores == N
    assert nloc % nblk == 0
    blk_sub = nloc // nblk          # rows each core contributes to a block
    blk_rows = blk_sub * n_cores    # rows of one table block
    assert blk_rows < 32768, "dma_gather idx is int16"
    nwin = math.ceil(nloc / win)

    deg = np.bincount(dst, minlength=N).astype(np.float64) + 1.0
    dinv = (1.0 / np.sqrt(deg)).astype(np.float32)

    loop = np.arange(N, dtype=np.int64)
    s_all = np.concatenate([src, loop])
    d_all = np.concatenate([dst, loop])

    # src -> (block, row inside block); block k holds rows
    # [k*blk_sub, (k+1)*blk_sub) of every core's shard, in rank order
    # (matches the chunked AllGather output layout, both layers).
    s_core = s_all // nloc
    s_rem = s_all % nloc
    s_blk = s_rem // blk_sub
    s_idx = (s_core * blk_sub + s_rem % blk_sub).astype(np.int64)

    e_core = d_all // nloc
    ld = d_all % nloc
    e_win = ld // win
    e_slot = ld % win

    # tiles per (window, block): max over cores so the program is identical
    key = ((e_core * nwin + e_win) * nblk + s_blk).astype(np.int64)
    cnt = np.bincount(key, minlength=n_cores * nwin * nblk)
    cnt = cnt.reshape(n_cores, nwin, nblk)
    T_wb = -(-cnt.max(axis=0) // 128)           # [nwin, nblk]
    assert (T_wb[:, :2].sum(axis=1) > 0).all()
    assert (T_wb[:, 2:].sum(axis=1) > 0).all()

    tstart = np.zeros((nwin, nblk), np.int64)
    tstart[1:, :] = np.cumsum(T_wb[:-1, :], axis=0)
    blk_tiles = T_wb.sum(axis=0)                # [nblk]
    blk_off = np.concatenate([[0], np.cumsum(blk_tiles)])
    ntiles_tot = int(blk_off[-1])

    calls_blk = [int(math.ceil(blk_tiles[b] / tq)) for b in range(nblk)]
    icols_blk = [calls_blk[b] * tq * 8 for b in range(nblk)]
    icol_off = np.concatenate([[0], np.cumsum(icols_blk)]).astype(np.int64)
    icols_tot = int(icol_off[-1])

    m = Meta()
    m.n_cores, m.win, m.nblk, m.tq = n_cores, win, nblk, tq
    m.nloc, m.blk_sub, m.blk_rows, m.nwin = nloc, blk_sub, blk_rows, nwin
    m.N = N
    m.T_wb, m.tstart = T_wb, tstart
    m.blk_tiles, m.blk_off = blk_tiles, blk_off
    m.calls_blk, m.icol_off = calls_blk, icol_off
    m.ntiles_tot, m.icols_tot = ntiles_tot, icols_tot
    m.passes = [[0, 1], [2, 3]]

    gw = np.asarray(gate_W, np.float16)
    w1 = np.asarray(W1, np.float16)
    w2 = np.asarray(W2, np.float16)
    gb = np.asarray(gate_b, np.float32).reshape(D, 1)
    b1rep = np.tile(np.asarray(b1, np.float32).reshape(1, D), (128, 1))
    b2rep = np.tile(np.asarray(b2, np.float32).reshape(1, D), (128, 1))

    in_maps = []
    for c in range(n_cores):
        sel = np.nonzero(e_core == c)[0]
        eb = s_blk[sel]
        ew = e_win[sel]
        order = np.lexsort((ew, eb))
        sel = sel[order]
        eb = eb[order]
        ew = ew[order]
        es = s_idx[sel]
        eslot = e_slot[sel]

        gkey = eb * nwin + ew
        group_start = np.searchsorted(gkey, np.arange(nblk * nwin))
        rank = np.arange(len(gkey)) - group_start[gkey]
        tg = rank // 128
        p = rank % 128
        bt = tstart[ew, eb] + tg                 # tile index inside block
        col = blk_off[eb] + bt                   # global meta column
        assert (tg < T_wb[ew, eb]).all()

        edst = np.full((128, ntiles_tot), -1.0, np.float16)
        edst[p, col] = eslot.astype(np.float16)

        idx_cols = []
        for b in range(nblk):
            mask_b = eb == b
            flat = np.zeros(calls_blk[b] * tq * 128, np.int16)
            flat[(bt[mask_b] * 128 + p[mask_b])] = es[mask_b].astype(np.int16)
            for cidx in range(calls_blk[b]):
                v = flat[cidx * tq * 128:(cidx + 1) * tq * 128]
                idx_cols.append(v.reshape(tq * 8, 16).T)
        idx16 = np.concatenate(idx_cols, axis=1)
        assert idx16.shape == (16, icols_tot)
        idx16 = np.tile(idx16, (8, 1))

        dv = np.zeros((128, nwin), np.float32)
        dloc = dinv[c * nloc:(c + 1) * nloc]
        for w in range(nwin):
            s = dloc[w * win:(w + 1) * win]
            dv[:len(s), w] = s

        xs = x[c * nloc:(c + 1) * nloc]
        xT = np.ascontiguousarray(xs.T.astype(np.float16))
        xdT = np.ascontiguousarray(
            (xs * dloc[:, None]).T.astype(np.float16))

        in_maps.append({
            "xT": xT, "xdT": xdT,
            "gw": gw, "gbias": gb, "w1": w1, "b1rep": b1rep,
            "w2": w2, "b2rep": b2rep,
            "dinvw": dv,
            "eidx": np.ascontiguousarray(idx16),
            "edst": edst,
        })
    return in_maps, m


# --------------------------------------------------------------------------
# device program
# --------------------------------------------------------------------------

def _emit(tc, outs, ins, m):
    nc = tc.nc
    AG = mybir.AluOpType
    AF = mybir.ActivationFunctionType
    groups = [list(range(m.n_cores))]
    out_ap = outs["out"]

    def span(w):
        return min(m.win, m.nloc - w * m.win)

    # AllGather chunk k may fire once local window w is stored
    ag_after = {}
    for k in range(m.nblk):
        ag_after.setdefault(((k + 1) * m.blk_sub - 1) // m.win,
                            []).append(k)

    with (
        tc.tile_pool(name="sb", bufs=1) as sb,
        tc.tile_pool(name="ps", bufs=1, space="PSUM") as ps,
        tc.tile_pool(name="dr", bufs=1, space="DRAM") as dr,
    ):
        nc.gpsimd.load_library(library_config.mlp)

        # ---- constants / params ------------------------------------------
        ident16 = sb.tile([128, 128], F16, tag="id16")
        make_identity(nc, ident16[:, :])
        iota16 = sb.tile([128, 128], F16, tag="iota")
        nc.gpsimd.iota(iota16[:, :], pattern=[[1, 128]], base=0,
                       channel_multiplier=0,
                       allow_small_or_imprecise_dtypes=True)

        wgate = sb.tile([128, 128], F16, tag="wgate")
        nc.sync.dma_start(wgate[:, :], ins["gw"][:, :])
        w1sb = sb.tile([128, 128], F16, tag="w1sb")
        nc.sync.dma_start(w1sb[:, :], ins["w1"][:, :])
        w2sb = sb.tile([128, 128], F16, tag="w2sb")
        nc.sync.dma_start(w2sb[:, :], ins["w2"][:, :])
        gbias = sb.tile([128, 1], F32, tag="gbias")
        nc.sync.dma_start(gbias[:, :], ins["gbias"][:, :])
        b1rep = sb.tile([128, 128], F32, tag="b1rep")
        nc.sync.dma_start(b1rep[:, :], ins["b1rep"][:, :])
        b2rep = sb.tile([128, 128], F32, tag="b2rep")
        nc.sync.dma_start(b2rep[:, :], ins["b2rep"][:, :])
        dinvw = sb.tile([128, m.nwin], F32, tag="dinvw")
        nc.sync.dma_start(dinvw[:, :], ins["dinvw"][:, :])

        # ---- resident edge metadata --------------------------------------
        dst_sb = sb.tile([128, m.ntiles_tot], F16, tag="dst_sb")
        nc.sync.dma_start(dst_sb[:, :], ins["edst"][:, :])

        h1T = sb.tile([128, m.nloc], F16, tag="h1T")
        accT = sb.tile([128, m.nwin, 128], F32, tag="accT")

        # ---- DRAM scratch -------------------------------------------------
        g1_loc = dr.tile([m.nloc, 128], F16, tag="g1_loc")
        g2_loc = dr.tile([m.nloc, 128], F16, tag="g2_loc")
        g1_full = [dr.tile([m.blk_rows, 128], F16, tag=f"g1_full{k}",
                           name=f"g1_full{k}", addr_space="Shared")
                   for k in range(m.nblk)]
        g2_full = [dr.tile([m.blk_rows, 128], F16, tag=f"g2_full{k}",
                           name=f"g2_full{k}", addr_space="Shared")
                   for k in range(m.nblk)]

        def fire_ag(w, g_loc, g_full):
            for k in ag_after.get(w, []):
                nc.gpsimd.collective_compute(
                    "AllGather", AG.bypass, replica_groups=groups,
                    ins=[g_loc[k * m.blk_sub:(k + 1) * m.blk_sub, :]],
                    outs=[g_full[k][:, :]],
                )

        # ---- phase A: layer-1 dense (local shard), row-major g~1 ---------
        for w in range(m.nwin):
            cols = span(w)
            c0 = w * m.win
            xt = sb.tile([128, 128], F16, tag="xt", bufs=3)
            nc.sync.dma_start(xt[:, :cols], ins["xT"][:, c0:c0 + cols])
            xdt = sb.tile([128, 128], F16, tag="xdt", bufs=3)
            nc.sync.dma_start(xdt[:, :cols], ins["xdT"][:, c0:c0 + cols])
            pg = ps.tile([128, 128], F32, tag="pg", bufs=2)
            nc.tensor.matmul(pg[:, :cols], lhsT=wgate[:, :], rhs=xt[:, :cols],
                             start=True, stop=True)
            gt = sb.tile([128, 128], F16, tag="gt", bufs=2)
            nc.scalar.activation(gt[:, :cols], pg[:, :cols], AF.Sigmoid,
                                 bias=gbias[:, :])
            h0 = sb.tile([128, 128], F16, tag="h0", bufs=2)
            nc.vector.tensor_tensor(out=h0[:, :cols], in0=xdt[:, :cols],
                                    in1=gt[:, :cols], op=AG.mult)
            rp = ps.tile([128, 128], F32, tag="rp", bufs=2)
            nc.tensor.matmul(rp[:cols, :], lhsT=h0[:, :cols], rhs=w1sb[:, :],
                             start=True, stop=True)
            rc = sb.tile([128, 128], F16, tag="rc", bufs=2)
            nc.scalar.copy(rc[:cols, :], rp[:cols, :])
            nc.sync.dma_start(g1_loc[c0:c0 + cols, :], rc[:cols, :])
            fire_ag(w, g1_loc, g1_full)

        # ---- sparse phase ------------------------------------------------
        IGRP = 4  # idx cols loaded per DMA, in units of gather calls
        qctr = [0]   # round-robin SWDGE queue counter (shared both layers)

        def spmm(tables, flush, lag):
            gbufs = {}
            sbufs = {}
            idxbufs = {}
            pending = []

            def idx_slice(b, call):
                grp = call // IGRP
                if (b, grp) not in idxbufs:
                    ic0 = int(m.icol_off[b]) + grp * IGRP * m.tq * 8
                    cols = min(IGRP * m.tq * 8,
                               int(m.icol_off[b + 1]) - ic0)
                    buf = sb.tile([128, IGRP * m.tq * 8], I16, tag="idxb",
                                  bufs=4, name=f"idxb{b}_{grp}")
                    nc.sync.dma_start(buf[:, :cols],
                                      ins["eidx"][:, ic0:ic0 + cols])
                    idxbufs[(b, grp)] = buf
                off = (call % IGRP) * m.tq * 8
                return idxbufs[(b, grp)], off

            def ensure_call(b, call):
                if (b, call) in gbufs:
                    return
                ntile = int(min(m.tq, m.blk_tiles[b] - call * m.tq))
                gbuf = sb.tile([128, m.tq, 128], F16, tag="gbuf",
                               bufs=5, name=f"gbuf{b}_{call}")
                nidx = ntile * 128
                ibuf, ioff = idx_slice(b, call)
                nc.gpsimd.dma_gather(
                    gbuf[:, :ntile, :], tables[b][:, :],
                    ibuf[:, ioff:ioff + ntile * 8], nidx, nidx, 128,
                    single_packet=(nidx * 2 <= 4096),
                    queue_num=qctr[0] % 4)
                qctr[0] += 1
                # one-hot S for all tiles of this call:
                # S[p, t, col] = (edst[p, t0+t] == iota[col])
                t0 = int(m.blk_off[b]) + call * m.tq
                sbuf = sb.tile([128, m.tq, 128], F16, tag="sbuf",
                               bufs=5, name=f"sbuf{b}_{call}")
                nc.vector.tensor_tensor(
                    out=sbuf[:, :ntile, :],
                    in0=dst_sb[:, t0:t0 + ntile].unsqueeze(2)
                        .broadcast_to([128, ntile, 128]),
                    in1=iota16[:, :].unsqueeze(1)
                        .broadcast_to([128, ntile, 128]),
                    op=AG.is_equal)
                gbufs[(b, call)] = gbuf
                sbufs[(b, call)] = sbuf

            for p, blocks in enumerate(m.passes):
                for w in range(m.nwin):
                    nmm = sum(int(m.T_wb[w, b]) for b in blocks)
                    if nmm == 0:
                        continue
                    cols = span(w)
                    for b in blocks:
                        if m.T_wb[w, b] == 0:
                            continue
                        t0 = int(m.tstart[w, b])
                        t1 = t0 + int(m.T_wb[w, b])
                        for call in range(t0 // m.tq, (t1 - 1) // m.tq + 1):
                            ensure_call(b, call)
                    psw = ps.tile([128, 128], F32, tag="win", bufs=2)
                    k = 0
                    for b in blocks:
                        t0 = int(m.tstart[w, b])
                        for t in range(int(m.T_wb[w, b])):
                            bt = t0 + t
                            call = bt // m.tq
                            ti = bt % m.tq
                            nc.tensor.matmul(
                                psw[:cols, :],
                                lhsT=sbufs[(b, call)][:, ti, :cols],
                                rhs=gbufs[(b, call)][:, ti, :],
                                start=(k == 0), stop=(k == nmm - 1))
                            k += 1
                    dv = dinvw[:cols, w:w + 1]
                    if p == 0:
                        nc.vector.tensor_scalar(
                            accT[:cols, w, :], psw[:cols, :], dv, None,
                            op0=AG.mult)
                    else:
                        st2 = flush(w, cols, psw, dv)
                        if st2 is not None:
                            pending.append(st2)
                            if len(pending) > lag:
                                pending.pop(0)()
            for fn in pending:
                fn()

        # ---- layer 1 flush: h1 = relu(dinv*sum + b1); h~1 = dinv*h1 ------
        # stage 2 (PE transpose + layer-2 dense + AllGather) runs lagged so
        # it never blocks the PE sparse stream at the queue head.
        def flush1(w, cols, psw, dv):
            t1 = sb.tile([128, 128], F32, tag="fl_t1", bufs=2)
            nc.vector.scalar_tensor_tensor(
                out=t1[:cols, :], in0=psw[:cols, :], scalar=dv,
                in1=accT[:cols, w, :], op0=AG.mult, op1=AG.add)
            t2 = sb.tile([128, 128], F32, tag="fl_t2", bufs=2)
            nc.vector.tensor_tensor(out=t2[:cols, :], in0=t1[:cols, :],
                                    in1=b1rep[:cols, :], op=AG.add)
            hrow = sb.tile([128, 128], F16, tag="hrow", bufs=8)
            nc.vector.tensor_scalar(hrow[:cols, :], t2[:cols, :], dv, 0.0,
                                    op0=AG.mult, op1=AG.max)

            def stage2():
                tp = ps.tile([128, 128], F16, tag="tp", bufs=1)
                nc.tensor.transpose(tp[:, :cols], hrow[:cols, :],
                                    ident16[:cols, :cols])
                nc.scalar.copy(h1T[:, w * m.win:w * m.win + cols],
                               tp[:, :cols])
                p2 = ps.tile([128, 128], F32, tag="p2", bufs=1)
                nc.tensor.matmul(p2[:cols, :],
                                 lhsT=h1T[:, w * m.win:w * m.win + cols],
                                 rhs=w2sb[:, :], start=True, stop=True)
                g2c = sb.tile([128, 128], F16, tag="g2c", bufs=2)
                nc.scalar.copy(g2c[:cols, :], p2[:cols, :])
                nc.sync.dma_start(g2_loc[w * m.win:w * m.win + cols, :],
                                  g2c[:cols, :])
                fire_ag(w, g2_loc, g2_full)
            return stage2

        spmm(g1_full, flush1, lag=4)

        # ---- layer 2 sparse + final flush --------------------------------
        def flush2(w, cols, psw, dv):
            t1 = sb.tile([128, 128], F32, tag="f2_t1", bufs=2)
            nc.vector.scalar_tensor_tensor(
                out=t1[:cols, :], in0=psw[:cols, :], scalar=dv,
                in1=accT[:cols, w, :], op0=AG.mult, op1=AG.add)
            t2 = sb.tile([128, 128], F32, tag="f2_t2", bufs=2)
            nc.vector.tensor_tensor(out=t2[:cols, :], in0=t1[:cols, :],
                                    in1=b2rep[:cols, :], op=AG.add)
            nc.sync.dma_start(out_ap[w * m.win:w * m.win + cols, :],
                              t2[:cols, :])
            return None

        spmm(g2_full, flush2, lag=0)


def declare_io(nc, m):
    ins = {
        "xT": nc.dram_tensor("xT", [D, m.nloc], F16, kind="ExternalInput").ap(),
        "xdT": nc.dram_tensor("xdT", [D, m.nloc], F16,
                              kind="ExternalInput").ap(),
        "gw": nc.dram_tensor("gw", [D, D], F16, kind="ExternalInput").ap(),
        "gbias": nc.dram_tensor("gbias", [D, 1], F32, kind="ExternalInput").ap(),
        "w1": nc.dram_tensor("w1", [D, D], F16, kind="ExternalInput").ap(),
        "b1rep": nc.dram_tensor("b1rep", [D, D], F32, kind="ExternalInput").ap(),
        "w2": nc.dram_tensor("w2", [D, D], F16, kind="ExternalInput").ap(),
        "b2rep": nc.dram_tensor("b2rep", [D, D], F32, kind="ExternalInput").ap(),
        "dinvw": nc.dram_tensor("dinvw", [D, m.nwin], F32,
                                kind="ExternalInput").ap(),
        "eidx": nc.dram_tensor("eidx", [128, max(m.icols_tot, 1)], I16,
                               kind="ExternalInput").ap(),
        "edst": nc.dram_tensor("edst", [128, max(m.ntiles_tot, 1)], F16,
                               kind="ExternalInput").ap(),
    }
    outs = {
        "out": nc.dram_tensor("out", [m.nloc, D], F32,
                              kind="ExternalOutput").ap(),
    }
    return ins, outs


def _build(m):
    nc = bacc.Bacc("TRN2", target_bir_lowering=False, debug=False,
                   enable_asserts=False, num_devices=m.n_cores,
                   num_swdge_queues=4)
    ins, outs = declare_io(nc, m)
    with tile.TileContext(nc) as tc:
        _emit(tc, outs, ins, m)
    nc.compile()
    return nc


def kernel(**inputs):
    global LAST_RESULTS, LAST_NC, LAST_IN_MAPS, LAST_META
    in_maps, m = _prep(**inputs)
    nc = _build(m)
    LAST_NC, LAST_IN_MAPS, LAST_META = nc, in_maps, m
    res = run_bass_kernel_spmd(
        nc, in_maps, core_ids=list(range(m.n_cores)), trace=False)
    LAST_RESULTS = res
    out = np.concatenate([res.results[c]["out"] for c in range(m.n_cores)],
                         axis=0)
    return np.ascontiguousarray(out.astype(np.float32))
